# revision 37
# baseline (speedup 1.0000x reference)
"""MoE (8 experts, top-2, SwiGLU FFN) Trainium2 kernel — top-2 sparse.

Sharding: 4 expert-groups x 2 data-halves. Core c = (eg, dg) holds experts
{2eg, 2eg+1} and tokens [dg*2048, (dg+1)*2048). Each core routes its 2048
tokens (full fp32 router over all 8 experts), compacts the token lists of
its 2 local experts on-chip, gathers those tokens' x rows by indirect DMA,
runs the SwiGLU FFN only for the selected (token, expert) pairs (bf16
matmuls, ~4x less work than dense), and scatters cw*(y+b2) rows back to
token positions in a per-expert output region. The host sums the 4 expert
groups' regions per data half (pure gather/unshard arithmetic).

SPMD trick: the router weight columns are permuted per core so the local
experts are always logits columns 0 and 1 — softmax and top-2 are
permutation-invariant, so one identical program serves all 8 cores.

On-chip compaction (no DRAM round-trip, no per-element DMA):
  mask m[t] = cw[t, e] > 0
  pos[t]   = exclusive-prefix-sum(m) + BIG*(1-m)     (matmul w/ triangular)
  P_tt[p,s] = (pos[p,tt] == s)                        (DVE vs iota row)
  [ids_hi; ids_lo; cw_g](s) = st[:,tt,:]^T @ P_tt     (accumulated matmul)
ids arrive as a [3, CAP] PSUM row; a 32x32 DVE block transpose + 4 tiny
partition-shift DMAs give [128, SB] per-partition offsets for the
indirect gather (x rows, bf16 2KB) and scatter (y rows, fp32 4KB,
bounds-checked so empty slots are dropped).

Capacity: CAP=640 per (core, expert); actual max for the seed-0 input is
540 (mean 512, sigma ~20). A `cnt` output lets the host assert no drop.
"""

import contextlib

import numpy as np

import concourse.bass as bass
import concourse.bacc as bacc
import concourse.mybir as mybir
import concourse.tile as tile
from concourse.masks import make_identity, make_upper_triangular

D, H, E, T = 1024, 512, 8, 4096
NCORES = 8
NEG = 4                     # expert groups
NDG = 2                     # data groups
ELOC = E // NEG             # 2 local experts per core
TC = T // NDG               # 2048 tokens per core
TT = TC // 128              # 16 token tiles
DT = D // 128               # 8 d-tiles
HT = H // 128               # 4 h-tiles
CAP = 640                   # per-(core,expert) token capacity
SB = CAP // 128             # 5 slot blocks
BIG = 1.0e5
N_WARM = 5
F32 = mybir.dt.float32
BF16 = mybir.dt.bfloat16
I32 = mybir.dt.int32
AX = mybir.AluOpType
ACT_EXP = mybir.ActivationFunctionType.Exp
ACT_SILU = mybir.ActivationFunctionType.Silu


def _bc(ap, n):
    return ap.broadcast_to([*ap.shape, n])


STAGE = 99   # debug: 1=router, 2=+compaction, 3=+gather/transpose, 99=full


def build_nc(loop_n=1):
    nc = _build_body(loop_n)
    nc.compile()
    return nc


def _build_body(loop_n=1):
    nc = bacc.Bacc("TRN2", target_bir_lowering=False, debug=False,
                   num_devices=NCORES)

    # tt-major x so the router pipeline starts after the first 512KB chunk
    xtf = nc.dram_tensor("xtf", [TT, DT, 128, 128], F32, kind="ExternalInput")
    xrow = nc.dram_tensor("xrow", [TC, D], BF16, kind="ExternalInput")
    rwt = nc.dram_tensor("rwt", [DT, 128, E], F32, kind="ExternalInput")
    w1t = nc.dram_tensor("w1t", [ELOC, DT, 128, H], BF16, kind="ExternalInput")
    w3t = nc.dram_tensor("w3t", [ELOC, DT, 128, H], BF16, kind="ExternalInput")
    w2t = nc.dram_tensor("w2t", [ELOC, HT, 128, D], BF16, kind="ExternalInput")
    b1t = nc.dram_tensor("b1t", [ELOC, HT, 128], F32, kind="ExternalInput")
    b3t = nc.dram_tensor("b3t", [ELOC, HT, 128], F32, kind="ExternalInput")
    b2t = nc.dram_tensor("b2t", [1, ELOC, D], F32, kind="ExternalInput")
    hilo = nc.dram_tensor("hilo", [128, TT, 2], BF16, kind="ExternalInput")
    iotas = nc.dram_tensor("iotas", [128, CAP], F32, kind="ExternalInput")
    osc0 = nc.dram_tensor("osc0", [TC, D], F32, kind="ExternalOutput")
    osc1 = nc.dram_tensor("osc1", [TC, D], F32, kind="ExternalOutput")
    cnt = nc.dram_tensor("cnt", [1, ELOC], F32, kind="ExternalOutput")
    osc = [osc0, osc1]

    with tile.TileContext(nc) as tc:
        with (
            tc.tile_pool(name="singles", bufs=1) as singles,
            tc.tile_pool(name="ppool", bufs=2) as ppool,
            tc.tile_pool(name="xgpool", bufs=2) as xgpool,
            tc.tile_pool(name="gpool", bufs=2) as gpool,
            tc.tile_pool(name="ypool", bufs=4) as ypool,
            tc.tile_pool(name="pmmA", bufs=4, space="PSUM") as pmmA,
            tc.tile_pool(name="pmmB", bufs=2, space="PSUM") as pmmB,
        ):
          def emit_once():
            # ---- one-time loads (issue order = DMA queue order) ------------
            rwt_sb = singles.tile([128, DT, E], F32)
            nc.sync.dma_start(out=rwt_sb, in_=rwt.ap().rearrange("a p e -> p a e"))
            hilo_sb = singles.tile([128, TT, 2], BF16)
            nc.sync.dma_start(out=hilo_sb, in_=hilo.ap())
            iota_sb = singles.tile([128, CAP], F32)
            nc.sync.dma_start(out=iota_sb, in_=iotas.ap())
            b1_sb = singles.tile([128, ELOC, HT], F32)
            nc.sync.dma_start(out=b1_sb, in_=b1t.ap().rearrange("e h p -> p e h"))
            b3_sb = singles.tile([128, ELOC, HT], F32)
            nc.sync.dma_start(out=b3_sb, in_=b3t.ap().rearrange("e h p -> p e h"))
            b2_sb = singles.tile([1, ELOC, D], F32)
            nc.sync.dma_start(out=b2_sb, in_=b2t.ap())
            xtf_sb = singles.tile([128, DT, TT, 128], F32)
            for tt in range(TT):
                nc.sync.dma_start(out=xtf_sb[:, :, tt, :],
                                  in_=xtf.ap()[tt].rearrange("a p b -> p a b"))
            w1_sb = singles.tile([128, ELOC, DT, H], BF16)
            w3_sb = singles.tile([128, ELOC, DT, H], BF16)
            w2_sb = singles.tile([128, ELOC, HT, D], BF16)
            for e in range(ELOC):
                nc.sync.dma_start(out=w1_sb[:, e],
                                  in_=w1t.ap()[e].rearrange("a p h -> p a h"))
                nc.sync.dma_start(out=w3_sb[:, e],
                                  in_=w3t.ap()[e].rearrange("a p h -> p a h"))
                nc.sync.dma_start(out=w2_sb[:, e],
                                  in_=w2t.ap()[e].rearrange("a p d -> p a d"))

            # on-chip constants (gpsimd, no DMA dep)
            utri = singles.tile([128, 128], F32)       # strict upper: 1 if r<c
            make_upper_triangular(nc, utri, val=1.0, diag=False)
            ident = singles.tile([128, 128], BF16)
            make_identity(nc, ident)
            ones_c = singles.tile([128, 1], F32)
            nc.vector.memset(ones_c, 1.0)
            ones_r = singles.tile([1, 128], F32)
            nc.vector.memset(ones_r, 1.0)
            warm_sb = singles.tile([128, 512], BF16)
            nc.vector.memset(warm_sb, 0.0)

            # ---- PE warm-up ------------------------------------------------
            p_warm = pmmA.tile([128, 512], F32, tag="mm")
            for _ in range(N_WARM):
                nc.tensor.matmul(p_warm, warm_sb[:, 0:128], warm_sb,
                                 start=True, stop=True)

            # ---- router: logits[t, e] per t-tile, full fp32 ----------------
            lg_sb = singles.tile([128, TT, E], F32)
            for tt in range(TT):
                p_sm = pmmB.tile([128, 128], F32, tag="sm")
                p_lg = p_sm[:, 0:E]
                for dt in range(DT):
                    nc.tensor.matmul(p_lg, xtf_sb[:, dt, tt, :],
                                     rwt_sb[:, dt, :],
                                     start=(dt == 0), stop=(dt == DT - 1))
                nc.vector.tensor_copy(lg_sb[:, tt, :], p_lg)

            # softmax over e (logits ~ N(0,1): no max-subtraction needed),
            # then top-2 via second-max thresholding; scores becomes cw.
            scores = singles.tile([128, TT, E], F32)
            nc.scalar.activation(scores, lg_sb, ACT_EXP)
            ssum = singles.tile([128, TT], F32)
            nc.vector.reduce_sum(ssum, scores, axis=mybir.AxisListType.X)
            nc.vector.reciprocal(ssum, ssum)
            nc.vector.tensor_tensor(scores, scores, _bc(ssum, E), op=AX.mult)
            m1 = singles.tile([128, TT], F32)
            nc.vector.reduce_max(m1, scores, axis=mybir.AxisListType.X)
            tmp_e = singles.tile([128, TT, E], F32)
            nc.vector.tensor_tensor(tmp_e, scores, _bc(m1, E), op=AX.is_equal)
            nc.vector.scalar_tensor_tensor(tmp_e, tmp_e, -1e30, scores,
                                           op0=AX.mult, op1=AX.add)
            m2 = singles.tile([128, TT], F32)
            nc.vector.reduce_max(m2, tmp_e, axis=mybir.AxisListType.X)
            nc.vector.tensor_tensor(tmp_e, scores, _bc(m2, E), op=AX.is_ge)
            nc.vector.tensor_tensor(scores, scores, tmp_e, op=AX.mult)

            if STAGE < 2:
                return
            # ---- per-local-expert compaction -------------------------------
            # cw for local expert e is scores[:, :, e] (host permuted rwt)
            cnt_sb = singles.tile([1, ELOC], F32)
            lcw_l, idg_l, ids_l = [], [], []
            for e in range(ELOC):
                m_e = singles.tile([128, TT], F32)
                nc.vector.tensor_scalar(m_e, scores[:, :, e], 0.0, scalar2=None,
                                        op0=AX.is_gt)
                # per-tile totals -> [16, 1]
                p_cs_t = pmmB.tile([128, 128], F32, tag="sm")
                p_cs = p_cs_t[0:16, 0:1]
                nc.tensor.matmul(p_cs, m_e, ones_c, start=True, stop=True)
                cs_sb = singles.tile([16, 1], F32)
                nc.vector.tensor_copy(cs_sb, p_cs)
                # total count (for host-side overflow assert)
                p_cnt_t = pmmB.tile([128, 128], F32, tag="sm")
                p_cnt = p_cnt_t[0:1, 0:1]
                nc.tensor.matmul(p_cnt, cs_sb, ones_c[0:16], start=True,
                                 stop=True)
                nc.vector.tensor_copy(cnt_sb[:, e:e + 1], p_cnt)
                # exclusive prefix over tiles -> [16, 1]
                p_off_t = pmmB.tile([128, 128], F32, tag="sm")
                p_off = p_off_t[0:16, 0:1]
                nc.tensor.matmul(p_off, utri[0:16, 0:16], cs_sb, start=True,
                                 stop=True)
                off_sb = singles.tile([32, 32], F32)
                nc.vector.memset(off_sb, 0.0)
                nc.vector.tensor_copy(off_sb[0:16, 0:1], p_off)
                offT = singles.tile([32, 32], F32)
                nc.vector.transpose(offT, off_sb)          # row 0 = offsets
                # pos_psum = within-tile excl prefix + replicated tile offset
                p_pf_t = pmmB.tile([128, 128], F32, tag="sm")
                p_pf = p_pf_t[:, 0:TT]
                nc.tensor.matmul(p_pf, utri, m_e, start=True, stop=False)
                nc.tensor.matmul(p_pf, ones_r, offT[0:1, 0:TT], start=False,
                                 stop=True)
                # pos = pos_psum + BIG*(1 - m)
                pos = singles.tile([128, TT], F32)
                nc.vector.scalar_tensor_tensor(pos, m_e, -BIG, p_pf,
                                               op0=AX.mult, op1=AX.add)
                nc.vector.tensor_scalar_add(pos, pos, BIG)

                # stationary [128, TT, 3]: (id_hi, id_lo, cw) in bf16
                st = singles.tile([128, TT, 3], BF16)
                nc.vector.tensor_copy(st[:, :, 0:2], hilo_sb)
                nc.vector.tensor_copy(st[:, :, 2], scores[:, :, e])
                # accumulate [3, CAP] = sum_tt st[:,tt,:]^T @ P_tt
                p_ap0_t = pmmA.tile([128, 512], F32, tag="mm")
                p_ap0 = p_ap0_t[0:3, :]
                p_ap1_t = pmmB.tile([128, 128], F32, tag="sm")
                p_ap1 = p_ap1_t[0:3, :]
                for tt in range(TT):
                    p_tt = ppool.tile([128, CAP], BF16, tag="P")
                    nc.vector.tensor_scalar(p_tt, iota_sb, pos[:, tt:tt + 1],
                                            scalar2=None, op0=AX.is_equal)
                    nc.tensor.matmul(p_ap0, st[:, tt, :], p_tt[:, 0:512],
                                     start=(tt == 0), stop=(tt == TT - 1))
                    nc.tensor.matmul(p_ap1, st[:, tt, :], p_tt[:, 512:CAP],
                                     start=(tt == 0), stop=(tt == TT - 1))
                # [3, CAP] -> [128, SB, 3] via 32x32 transpose + 4 shifts
                idrow = singles.tile([32, CAP], F32)
                nc.vector.memset(idrow, 0.0)
                nc.vector.tensor_copy(idrow[0:3, 0:512], p_ap0)
                nc.vector.tensor_copy(idrow[0:3, 512:CAP], p_ap1)
                idt = singles.tile([32, CAP // 32, 32], F32)
                nc.vector.transpose(idt.rearrange("p a b -> p (a b)"), idrow)
                lcw = singles.tile([128, SB, 3], F32)
                for q in range(4):
                    nc.gpsimd.dma_start(
                        out=lcw[32 * q:32 * (q + 1), :, :],
                        in_=idt[:, q::4, 0:3])
                # gather ids (hi+lo), scatter ids (+BIG where empty slot)
                idg_f = singles.tile([128, SB], F32)
                nc.vector.tensor_tensor(idg_f, lcw[:, :, 0], lcw[:, :, 1],
                                        op=AX.add)
                idg = singles.tile([128, SB], I32)
                nc.vector.tensor_copy(idg, idg_f)
                ids_f = singles.tile([128, SB], F32)
                nc.vector.tensor_scalar(ids_f, lcw[:, :, 2], 0.0, scalar2=None,
                                        op0=AX.is_le)
                nc.vector.scalar_tensor_tensor(ids_f, ids_f, BIG, idg_f,
                                               op0=AX.mult, op1=AX.add)
                ids = singles.tile([128, SB], I32)
                nc.vector.tensor_copy(ids, ids_f)
                lcw_l.append(lcw)
                idg_l.append(idg)
                ids_l.append(ids)
            nc.sync.dma_start(out=cnt.ap(), in_=cnt_sb)

            if STAGE < 3:
                return
            # ---- b2 broadcast tiles [128, ELOC, D] -------------------------
            b2bc = singles.tile([128, ELOC, D], F32)
            for e in range(ELOC):
                for dc in range(2):
                    p_b = pmmA.tile([128, 512], F32, tag="mm")
                    nc.tensor.matmul(p_b, ones_r,
                                     b2_sb[:, e, dc * 512:(dc + 1) * 512],
                                     start=True, stop=True)
                    nc.vector.tensor_copy(b2bc[:, e, dc * 512:(dc + 1) * 512],
                                          p_b)

            # ---- gather x rows for both experts ----------------------------
            xtg_l = []
            for e in range(ELOC):
                xtg = singles.tile([128, DT, CAP], BF16)
                xtg_l.append(xtg)
                for sb in range(SB):
                    xg = xgpool.tile([128, D], BF16, tag="xg")
                    nc.gpsimd.indirect_dma_start(
                        out=xg, out_offset=None, in_=xrow.ap(),
                        in_offset=bass.IndirectOffsetOnAxis(
                            ap=idg_l[e][:, sb:sb + 1], axis=0))
                    for dt in range(DT):
                        p_tr = pmmB.tile([128, 128], BF16, tag="tr")
                        nc.tensor.transpose(
                            p_tr, xg[:, dt * 128:(dt + 1) * 128], ident)
                        nc.vector.tensor_copy(
                            xtg[:, dt, sb * 128:(sb + 1) * 128], p_tr)

            if STAGE < 4:
                return
            # ---- sparse SwiGLU FFN + scatter-combine -----------------------
            chunks = [(0, 512), (512, CAP)]
            for e in range(ELOC):
                xtg = xtg_l[e]
                gu = gpool.tile([128, HT, CAP], BF16, tag="gu")
                for ht in range(HT):
                    hs = slice(ht * 128, (ht + 1) * 128)
                    for c0, c1 in chunks:
                        cs = slice(c0, c1)
                        tag = "mm" if c1 - c0 == 512 else "sm"
                        pool = pmmA if c1 - c0 == 512 else pmmB
                        p_h = pool.tile([128, c1 - c0], F32, tag=tag)
                        for dt in range(DT):
                            nc.tensor.matmul(p_h, w1_sb[:, e, dt, hs],
                                             xtg[:, dt, cs],
                                             start=(dt == 0),
                                             stop=(dt == DT - 1))
                        g_t = gpool.tile([128, CAP], F32, tag="g")
                        nc.scalar.activation(g_t[:, cs], p_h, ACT_SILU,
                                             bias=b1_sb[:, e, ht:ht + 1],
                                             scale=1.0)
                        p_u = pool.tile([128, c1 - c0], F32, tag=tag)
                        for dt in range(DT):
                            nc.tensor.matmul(p_u, w3_sb[:, e, dt, hs],
                                             xtg[:, dt, cs],
                                             start=(dt == 0),
                                             stop=(dt == DT - 1))
                        nc.vector.scalar_tensor_tensor(
                            gu[:, ht, cs], p_u, b3_sb[:, e, ht:ht + 1],
                            g_t[:, cs], op0=AX.add, op1=AX.mult)
                for tb in range(SB):
                    ts_ = slice(tb * 128, (tb + 1) * 128)
                    y_t = ypool.tile([128, D], F32, tag="y")
                    for dc in range(2):
                        ds_ = slice(dc * 512, (dc + 1) * 512)
                        p_y = pmmA.tile([128, 512], F32, tag="mm")
                        for ht in range(HT):
                            nc.tensor.matmul(p_y, gu[:, ht, ts_],
                                             w2_sb[:, e, ht, ds_],
                                             start=(ht == 0),
                                             stop=(ht == HT - 1))
                        nc.vector.tensor_tensor(y_t[:, ds_], p_y,
                                                b2bc[:, e, ds_], op=AX.add)
                        nc.vector.tensor_scalar(
                            y_t[:, ds_], y_t[:, ds_],
                            lcw_l[e][:, tb:tb + 1, 2], scalar2=None,
                            op0=AX.mult)
                    nc.gpsimd.indirect_dma_start(
                        out=osc[e].ap(), out_offset=bass.IndirectOffsetOnAxis(
                            ap=ids_l[e][:, tb:tb + 1], axis=0),
                        in_=y_t, in_offset=None,
                        bounds_check=TC - 1, oob_is_err=False)

          for _ in range(loop_n):
              emit_once()

    return nc


_NC_CACHE = None


def _get_nc():
    global _NC_CACHE
    if _NC_CACHE is None:
        _NC_CACHE = build_nc()
    return _NC_CACHE


def make_in_maps(x, router_w, w1, b1, w3, b3, w2, b2):
    import ml_dtypes
    bf16 = ml_dtypes.bfloat16
    xt_full = np.ascontiguousarray(x.reshape(T, D)).astype(np.float32)

    t_ids = (np.arange(TT * 128).reshape(TT, 128).T)       # [128, TT]
    hilo = np.stack([(t_ids // 16) * 16, t_ids % 16], axis=-1)
    hilo = hilo.astype(bf16)                               # [128, TT, 2]
    iotas = np.broadcast_to(np.arange(CAP, dtype=np.float32), (128, CAP))
    iotas = np.ascontiguousarray(iotas)

    in_maps = []
    for c in range(NCORES):
        eg, dg = c // NDG, c % NDG
        el = [2 * eg, 2 * eg + 1]
        perm = el + [i for i in range(E) if i not in el]
        xs = xt_full[dg * TC:(dg + 1) * TC]                # [TC, D]
        xtf = np.ascontiguousarray(
            xs.T.reshape(DT, 128, TT, 128).transpose(2, 0, 1, 3))
        in_maps.append({
            "xtf": xtf,
            "xrow": np.ascontiguousarray(xs).astype(bf16),
            "rwt": np.ascontiguousarray(router_w[perm].T.astype(np.float32)
                                        ).reshape(DT, 128, E),
            "w1t": np.ascontiguousarray(w1[el].transpose(0, 2, 1)
                                        ).reshape(ELOC, DT, 128, H).astype(bf16),
            "w3t": np.ascontiguousarray(w3[el].transpose(0, 2, 1)
                                        ).reshape(ELOC, DT, 128, H).astype(bf16),
            "w2t": np.ascontiguousarray(w2[el].transpose(0, 2, 1)
                                        ).reshape(ELOC, HT, 128, D).astype(bf16),
            "b1t": np.ascontiguousarray(b1[el]).reshape(ELOC, HT, 128
                                                        ).astype(np.float32),
            "b3t": np.ascontiguousarray(b3[el]).reshape(ELOC, HT, 128
                                                        ).astype(np.float32),
            "b2t": np.ascontiguousarray(b2[el]).astype(np.float32)[None],
            "hilo": hilo,
            "iotas": iotas,
        })
    return in_maps


def assemble_output(per_core):
    """per_core: list of dicts with 'osc0', 'osc1', 'cnt' per core."""
    out = np.zeros((T, D), np.float32)
    for c in range(NCORES):
        dg = c % NDG
        cnts = np.asarray(per_core[c]["cnt"]).ravel()
        assert cnts.max() <= CAP, f"capacity overflow: {cnts}"
        sl = slice(dg * TC, (dg + 1) * TC)
        out[sl] += np.asarray(per_core[c]["osc0"])
        out[sl] += np.asarray(per_core[c]["osc1"])
    return out.reshape(4, 1024, D)


def kernel(x, router_w, w1, b1, w3, b3, w2, b2):
    from concourse.bass_utils import run_bass_kernel_spmd

    nc = _get_nc()
    args = [np.asarray(a, dtype=np.float32)
            for a in (x, router_w, w1, b1, w3, b3, w2, b2)]
    in_maps = make_in_maps(*args)
    res = run_bass_kernel_spmd(nc, in_maps, core_ids=list(range(NCORES)))
    return assemble_output(res.results)


# revision 40
# speedup vs baseline: 14.0539x; 14.0539x over previous
"""MoE (8 experts, top-2, SwiGLU FFN) Trainium2 kernel.

Sharding: data-parallel over tokens. Each of the 8 cores gets T/8 = 512
tokens and computes the full MoE for them: router (fp32 matmul + softmax +
top-2 via max/second-max thresholding) and all 8 experts' FFNs (fp32r
matmuls), accumulating cw-weighted expert outputs on-chip. Host only
reshapes/transposes inputs and concatenates the 8 output slices.

Schedule notes (cost-model driven):
 - A few discarded f32r matmuls warm the PE (HAM ramp) before the fp32
   router so the router runs at full clock (853ns vs 2429ns per matmul).
 - DMA issue order: rwt, x (per-d-tile chunks), b2, b1, then per-expert
   w1, (b3,) w3, w2 — so the first matmuls of each stage start as soon as
   their first operand lands.
 - The router->combine-weight chain (transpose, softmax, top-2) runs
   entirely on DVE/ACT (32x32 stream transposes + 4 tiny partition-shift
   DMAs on the gpsimd queue), so the PE stream never interleaves with it.
 - Output is written per (t-tile, d-chunk) to a DRAM-contiguous buffer;
   the host undoes the tiling permutation for free.

Layouts inside a core (partition dim first):
  xT      [128(d%128), 8(d//128), 512(t)]    moving operand of mm1/router
  w1T/w3T [128(d%128), 8(d//128), 512(h)]    stationary tiles [d,h] for mm1
  h/u     PSUM [128(h%128), 512(t)]          per h-tile, accum over d-tiles
  gu      [128(h%128), 4(h//128), 512(t)]    stationary tiles [h,t] for mm2
  w2T     [128(h%128), 4(h//128), 1024(d)]   moving operand of mm2
  y       PSUM [128(t%128), 512(d-chunk)]    accum over h-tiles
  out_acc [128(t%128), 4(t//128), 1024(d)]   sum_e cw_e * (y_e + b2_e)
"""

import numpy as np

import concourse.bass as bass
import concourse.bacc as bacc
import concourse.mybir as mybir
import concourse.tile as tile

D, H, E, T = 1024, 512, 8, 4096
NCORES = 8
TLOC = T // NCORES          # 512 tokens per core
DT = D // 128               # 8 d-tiles
HT = H // 128               # 4 h-tiles
TT = TLOC // 128            # 4 t-tiles
DC = D // 512               # 2 d-chunks for mm2 moving operand
N_WARM = 5                  # discarded matmuls to ramp the PE clock
F32 = mybir.dt.float32
F32R = mybir.dt.float32r
AX = mybir.AluOpType


def _bc(ap, n):
    """Append a step-0 (broadcast) innermost free dim of size n."""
    return ap.broadcast_to([*ap.shape, n])


def build_nc(loop_n=1):
    nc = bacc.Bacc("TRN2", target_bir_lowering=False, debug=False,
                   num_devices=NCORES)

    xtf = nc.dram_tensor("xtf", [DT, 128, TLOC], F32, kind="ExternalInput")
    rwt = nc.dram_tensor("rwt", [DT, 128, E], F32, kind="ExternalInput")
    w1t = nc.dram_tensor("w1t", [E, DT, 128, H], F32R, kind="ExternalInput")
    w3t = nc.dram_tensor("w3t", [E, DT, 128, H], F32R, kind="ExternalInput")
    w2t = nc.dram_tensor("w2t", [E, HT, 128, D], F32R, kind="ExternalInput")
    b1t = nc.dram_tensor("b1t", [E, HT, 128], F32, kind="ExternalInput")
    b3t = nc.dram_tensor("b3t", [E, HT, 128], F32, kind="ExternalInput")
    b2 = nc.dram_tensor("b2", [E, D], F32R, kind="ExternalInput")
    out = nc.dram_tensor("out", [TT, DC, 128, 512], F32, kind="ExternalOutput")

    import contextlib
    with tile.TileContext(nc) as tc:
        # loop_n > 1 replays the identical body via a hardware loop — used
        # only by test.py's loop-differencing timer (kernel() uses loop_n=1).
        loop_cm = (tc.For_i(0, loop_n, 1) if loop_n > 1
                   else contextlib.nullcontext())
        with (
            tc.tile_pool(name="singles", bufs=1) as singles,
            tc.tile_pool(name="wpool", bufs=2) as wpool,
            tc.tile_pool(name="gpool", bufs=2) as gpool,
            tc.tile_pool(name="pmm", bufs=6, space="PSUM") as pmm,
            tc.tile_pool(name="psmall", bufs=2, space="PSUM") as psmall,
            loop_cm,
        ):
            # ---- one-time loads (order = DMA queue order) ------------------
            rwt_sb = singles.tile([128, DT, E], F32)
            nc.sync.dma_start(out=rwt_sb, in_=rwt.ap().rearrange("a p e -> p a e"))
            # x lands once as fp32 (router needs true fp32); the f32r FFN
            # copy is made on-chip by the otherwise-idle DVE (saves 2MB HBM)
            xtf_sb = singles.tile([128, DT, TLOC], F32)
            xtf_r = xtf.ap().rearrange("a p t -> p a t")
            for dt in range(DT):
                nc.sync.dma_start(out=xtf_sb[:, dt, :], in_=xtf_r[:, dt, :])
            xt_sb = singles.tile([128, DT, TLOC], F32R)
            for dt in range(DT):
                nc.vector.tensor_copy(xt_sb[:, dt, :], xtf_sb[:, dt, :])
            b2_sb = singles.tile([E, D], F32R)
            nc.sync.dma_start(out=b2_sb, in_=b2.ap())
            b1_sb = singles.tile([128, E, HT], F32)
            nc.sync.dma_start(out=b1_sb, in_=b1t.ap().rearrange("e h p -> p e h"))
            dume = singles.tile([1, 1], F32)
            nc.scalar.activation(dume, rwt_sb[0:1, 0, 0:1],
                                 mybir.ActivationFunctionType.Exp)

            # ---- PE warm-up: discarded f32r matmuls ------------------------
            p_warm = psmall.tile([128, TLOC], F32, tag="small")
            for _ in range(N_WARM):
                nc.tensor.matmul(p_warm, xt_sb[:, 0, 0:128], xt_sb[:, 0, :],
                                 start=True, stop=True)

            # ---- router: logitsT[e, t] = (router_w @ x.T) ------------------
            # full fp32 so top-2 selection matches the fp32 reference
            p_lg = psmall.tile([32, TLOC], F32, tag="small")
            nc.vector.memset(p_lg, 0.0)
            for dt in range(DT):
                nc.tensor.matmul(p_lg[0:E, :], rwt_sb[:, dt, :],
                                 xtf_sb[:, dt, :],
                                 start=(dt == 0), stop=(dt == DT - 1))
            # transpose logitsT straight out of PSUM on the DVE (32x32 block
            # transpose) so no PE op or copy sits in the router->cw chain
            lgT32 = singles.tile([32, 16, 32], F32)
            nc.vector.transpose(lgT32.rearrange("p a e -> p (a e)"), p_lg)
            # token t = 32*b + i lives at [i, b, e] for e < 8

            # softmax over e (no max-subtraction needed: logits ~ N(0,1));
            # scores32 doubles as the dense combine-weight tile (cols 8+ stay 0)
            sl = lgT32[:, :, 0:E]
            scores32 = singles.tile([32, 16, 32], F32)
            nc.vector.memset(scores32, 0.0)
            sc = scores32[:, :, 0:E]
            nc.scalar.activation(sc, sl, mybir.ActivationFunctionType.Exp)
            ssum = singles.tile([32, 16], F32)
            nc.vector.reduce_sum(ssum, sc, axis=mybir.AxisListType.X)
            rsum = singles.tile([32, 16], F32)
            nc.vector.reciprocal(rsum, ssum)
            nc.vector.tensor_tensor(sc, sc, _bc(rsum, E), op=AX.mult)

            # top-2: cw = score * (score >= second_max)
            m1 = singles.tile([32, 16], F32)
            nc.vector.reduce_max(m1, sc, axis=mybir.AxisListType.X)
            tmp32 = singles.tile([32, 16, E], F32)
            nc.vector.tensor_tensor(tmp32, sc, _bc(m1, E), op=AX.is_equal)
            nc.vector.scalar_tensor_tensor(tmp32, tmp32, -1e30, sc,
                                           op0=AX.mult, op1=AX.add)
            m2 = singles.tile([32, 16], F32)
            nc.vector.reduce_max(m2, tmp32, axis=mybir.AxisListType.X)
            nc.vector.tensor_tensor(tmp32, sc, _bc(m2, E), op=AX.is_ge)
            nc.vector.tensor_tensor(sc, sc, tmp32, op=AX.mult)

            # cwT[e, t] via a second DVE block transpose (rows 8+ are junk)
            cwTp = singles.tile([32, 16, 32], F32)
            nc.vector.transpose(cwTp.rearrange("p a e -> p (a e)"),
                                scores32.rearrange("p a e -> p (a e)"))
            cwT = singles.tile([E, 16, 32], F32R)
            nc.vector.tensor_copy(cwT, cwTp[0:E, :, :])

            # cw in [t%128, tt, e] layout for the y-combine scalars:
            # 4 tiny partition-shift DMAs (gpsimd queue; sync queue carries
            # the big weight streams and must not head-of-line block on cw)
            cw128 = singles.tile([128, TT, E], F32)
            cw_v = scores32.rearrange("p (t q) e -> p t q e", q=4)
            for q in range(4):
                nc.gpsimd.dma_start(out=cw128[32 * q:32 * (q + 1), :, :],
                                    in_=cw_v[:, :, q, 0:E])

            def emit_expert_hu(e, w1_sb, w3_sb, w2_sb):
                g_sb = gpool.tile([128, HT, TLOC], F32, tag="g")
                hb_sb = gpool.tile([128, HT, TLOC], F32, tag="hb")
                gu_sb = gpool.tile([128, HT, TLOC], F32R, tag="gu")
                for ht in range(HT):
                    hs = slice(ht * 128, (ht + 1) * 128)
                    p_h = pmm.tile([128, TLOC], F32, tag="mm")
                    for dt in range(DT):
                        nc.tensor.matmul(p_h, w1_sb[:, dt, hs], xt_sb[:, dt, :],
                                         start=(dt == 0), stop=(dt == DT - 1))
                    # silu(h+b1)*(u+b3) = (h+b1)*sigmoid(h+b1)*(u+b3)
                    nc.scalar.activation(g_sb[:, ht, :], p_h,
                                         mybir.ActivationFunctionType.Sigmoid,
                                         bias=b1_sb[:, e, ht:ht + 1], scale=1.0)
                    nc.vector.tensor_scalar_add(hb_sb[:, ht, :], p_h,
                                                b1_sb[:, e, ht:ht + 1])
                for ht in range(HT):
                    hs = slice(ht * 128, (ht + 1) * 128)
                    p_u = pmm.tile([128, TLOC], F32, tag="mm")
                    for dt in range(DT):
                        last_u = nc.tensor.matmul(p_u, w3_sb[:, dt, hs],
                                                  xt_sb[:, dt, :],
                                                  start=(dt == 0),
                                                  stop=(dt == DT - 1))
                    nc.vector.scalar_tensor_tensor(gu_sb[:, ht, :], p_u,
                                                   b3_sb[:, e, ht:ht + 1],
                                                   g_sb[:, ht, :],
                                                   op0=AX.add, op1=AX.mult)
                    nc.vector.tensor_mul(gu_sb[:, ht, :], gu_sb[:, ht, :],
                                         hb_sb[:, ht, :])
                return gu_sb, last_u

            def emit_expert_y(e, gu_sb, w2_sb):
                # y[t, d] = gu.T @ w2T ; out_acc += cw_e * y
                for tt in range(TT):
                    ts_ = slice(tt * 128, (tt + 1) * 128)
                    for dc in range(DC):
                        ds_ = slice(dc * 512, (dc + 1) * 512)
                        p_y = pmm.tile([128, 512], F32, tag="mm")
                        for ht in range(HT):
                            nc.tensor.matmul(p_y, gu_sb[:, ht, ts_],
                                             w2_sb[:, ht, ds_],
                                             start=(ht == 0), stop=(ht == HT - 1))
                        nc.vector.scalar_tensor_tensor(
                            out_acc[:, tt, ds_], p_y, cw128[:, tt, e:e + 1],
                            out_acc[:, tt, ds_], op0=AX.mult, op1=AX.add)

            def emit_expert_dmas(e):
                w1_sb = wpool.tile([128, DT, H], F32R, tag="w1")
                nc.sync.dma_start(out=w1_sb,
                                  in_=w1t.ap()[e].rearrange("a p h -> p a h"))
                if e == 0:
                    nc.sync.dma_start(out=b3_sb,
                                      in_=b3t.ap().rearrange("e h p -> p e h"))
                w3_sb = wpool.tile([128, DT, H], F32R, tag="w3")
                nc.sync.dma_start(out=w3_sb,
                                  in_=w3t.ap()[e].rearrange("a p h -> p a h"))
                w2_sb = wpool.tile([128, HT, D], F32R, tag="w2")
                nc.sync.dma_start(out=w2_sb,
                                  in_=w2t.ap()[e].rearrange("a p d -> p a d"))
                return w1_sb, w3_sb, w2_sb

            # out_acc = cw @ b2 (the bias part of the combine)
            b3_sb = singles.tile([128, E, HT], F32)
            out_acc = singles.tile([128, TT, D], F32)
            for tt in range(TT):
                for dc in range(DC):
                    p_b = pmm.tile([128, 512], F32, tag="mm")
                    nc.tensor.matmul(p_b, cwT[:, 4 * tt:4 * (tt + 1), :],
                                     b2_sb[:, dc * 512:(dc + 1) * 512])
                    nc.vector.tensor_copy(out_acc[:, tt, dc * 512:(dc + 1) * 512],
                                          p_b)

            for e in range(E):
                w1_sb, w3_sb, w2_sb = emit_expert_dmas(e)
                gu_sb, _ = emit_expert_hu(e, w1_sb, w3_sb, w2_sb)
                emit_expert_y(e, gu_sb, w2_sb)

            # ---- store (chunked + DRAM-contiguous; host re-lays-out) -------
            out_r = out.ap().rearrange("a b p d -> p a b d")
            for tt in range(TT):
                for dc in range(DC):
                    nc.sync.dma_start(out=out_r[:, tt, dc, :],
                                      in_=out_acc[:, tt,
                                                  dc * 512:(dc + 1) * 512])

    nc.compile()
    return nc


_NC_CACHE = None


def _get_nc():
    global _NC_CACHE
    if _NC_CACHE is None:
        _NC_CACHE = build_nc()
    return _NC_CACHE


def make_in_maps(x, router_w, w1, b1, w3, b3, w2, b2):
    xt_full = np.ascontiguousarray(x.reshape(T, D))
    shared = {
        "rwt": np.ascontiguousarray(router_w.T).reshape(DT, 128, E),
        "w1t": np.ascontiguousarray(w1.transpose(0, 2, 1)).reshape(E, DT, 128, H),
        "w3t": np.ascontiguousarray(w3.transpose(0, 2, 1)).reshape(E, DT, 128, H),
        "w2t": np.ascontiguousarray(w2.transpose(0, 2, 1)).reshape(E, HT, 128, D),
        "b1t": np.ascontiguousarray(b1).reshape(E, HT, 128),
        "b3t": np.ascontiguousarray(b3).reshape(E, HT, 128),
        "b2": np.ascontiguousarray(b2),
    }
    shared = {k: v.astype(np.float32, copy=False) for k, v in shared.items()}
    in_maps = []
    for c in range(NCORES):
        xc = xt_full[c * TLOC:(c + 1) * TLOC]
        xtc = np.ascontiguousarray(xc.T).reshape(DT, 128, TLOC)
        in_maps.append(dict(shared, xtf=xtc))
    return in_maps


def assemble_output(per_core):
    """per_core: list (one per core) of dicts with the 'out' array."""
    outs = [np.asarray(per_core[c]["out"]).transpose(0, 2, 1, 3
                                                     ).reshape(TLOC, D)
            for c in range(NCORES)]
    return np.concatenate(outs, axis=0).reshape(4, 1024, D)


def kernel(x, router_w, w1, b1, w3, b3, w2, b2):
    from concourse.bass_utils import run_bass_kernel_spmd

    nc = _get_nc()
    in_maps = make_in_maps(np.asarray(x, dtype=np.float32),
                           np.asarray(router_w, dtype=np.float32),
                           np.asarray(w1, dtype=np.float32),
                           np.asarray(b1, dtype=np.float32),
                           np.asarray(w3, dtype=np.float32),
                           np.asarray(b3, dtype=np.float32),
                           np.asarray(w2, dtype=np.float32),
                           np.asarray(b2, dtype=np.float32))
    res = run_bass_kernel_spmd(nc, in_maps, core_ids=list(range(NCORES)))
    return assemble_output(res.results)

